# revision 16
# baseline (speedup 1.0000x reference)
"""HSIC pairwise loss kernel for trn2 (8 NeuronCores), fp8 DoubleRow.

Math: reference builds K_c = (w^2 w^2T) * (E_c E_c^T), M_c = R K_c, and sums
tr(M_i M_j) over i<j. With F_c = w^2 * E_c (row scaling), R the centering
matrix (idempotent):
    tr(R K_i R K_j) = ||G_i^T G_j||_F^2,  G_c = F_c - colmean(F_c)
and with A_ij = F_i^T F_j, s_c = F_c^T 1:
    G_i^T G_j = A_ij - (1/n) s_i s_j^T
so loss = sum_{i<j} ||A_ij - s_i s_j^T / n||_F^2 / (n-1)^2.

Device work: the 45 A_ij blocks [256,256] (contraction over n=4096) at
half-chunk granularity (20 units of 128 cols). Each core loads 9 units
as fp8e4 (4.7 MB, host pre-scales w^2*X by a power-of-two and quantizes)
and runs 8 matmul windows per 256-row k-super-tile into 8 PSUM banks
using MatmulPerfMode.DoubleRow (2 fp8 k-rows per cycle). Column sums s
are taken on the host from the same quantized array, so the device does
matmuls only. Host assembles quadrants, applies the rank-1 centering
correction and the final scalar reduction in float64 (~3 MFLOP).
"""

import numpy as np
import ml_dtypes
from contextlib import ExitStack

import concourse.bass as bass
import concourse.tile as tile
from concourse import bacc, mybir
from concourse import bass_utils

N = 4096
KT = 16                      # k super-tiles of 256 rows (DoubleRow)
UNITS = 9                    # half-chunk units per core
ROW = UNITS * 128            # 1152 data cols
WARM_MMS = 24                # dummy matmuls to pre-warm the PE (HAM)

# 8 cores x 9 units (of 20 half-chunks); covers all 180 cross-parent
# half-pairs via the fixed window pattern below (found by search).
ASSIGN = [
    [0, 19, 9, 4, 3, 18, 13, 17, 1],
    [10, 5, 8, 18, 4, 12, 9, 16, 15],
    [5, 17, 12, 9, 18, 2, 7, 14, 11],
    [14, 7, 12, 11, 2, 13, 1, 15, 16],
    [0, 17, 3, 19, 4, 6, 12, 11, 15],
    [14, 2, 9, 16, 5, 3, 19, 0, 6],
    [17, 18, 13, 11, 1, 6, 5, 8, 10],
    [3, 2, 19, 0, 14, 4, 7, 10, 8],
]

# (stat_slot, moving_start_col, n_cols)
WINDOWS = [
    (0, 640, 512),
    (1, 640, 512),
    (2, 640, 512),
    (3, 640, 512),
    (4, 640, 512),
    (5, 768, 384),
    (6, 896, 256),
    (7, 1024, 128),
]
OUT_COLS = sum(w[2] for w in WINDOWS)   # 3328

_CACHE = {}


def _build():
    f32 = mybir.dt.float32
    f8 = mybir.dt.float8e4
    DR = mybir.MatmulPerfMode.DoubleRow
    nc = bacc.Bacc("TRN2", target_bir_lowering=False, debug=False,
                   num_devices=8)
    bf16 = mybir.dt.bfloat16
    # host pre-interleaves the two 128-row k-halves of each 256-row
    # super-tile into one 2304-byte partition line: x2[k*128+p, :] =
    # [F[k*256+p, :], F[k*256+128+p, :]] — DMA runs are contiguous.
    x = nc.dram_tensor("x", [KT * 128, 2 * ROW], f8,
                       kind="ExternalInput").ap()
    out = nc.dram_tensor("out", [128, OUT_COLS], bf16,
                         kind="ExternalOutput").ap()

    with tile.TileContext(nc) as tc:
        with ExitStack() as ctx:
            zpool = ctx.enter_context(tc.tile_pool(name="z", bufs=1))
            xpool = ctx.enter_context(tc.tile_pool(name="xs", bufs=1))
            psum = ctx.enter_context(tc.tile_pool(name="ps", bufs=1,
                                                  space="PSUM"))
            opool = ctx.enter_context(tc.tile_pool(name="o", bufs=1))

            ps = []
            for i, (_, _, nw) in enumerate(WINDOWS):
                pst = psum.tile([128, nw], f32, tag=f"ps{i}", name=f"ps{i}")
                ps.append(pst)

            # PE warm-up: dummy DoubleRow matmuls while the first input DMAs
            # are in flight (HAM needs ~3.4us of activity to unthrottle
            # 1.2 -> 2.4 GHz). Products land in ps[0] and are discarded
            # (the real start=True matmul resets it).
            zt = zpool.tile([128, 2, 128], f8, tag="zt")
            nc.vector.memset(zt[:], 0.0)
            for _ in range(WARM_MMS):
                nc.tensor.matmul(ps[0][:, 0:128], zt[:, :, :], zt[:, :, :],
                                 start=True, stop=True, perf_mode=DR)

            # batch input DMAs 1+5+5+5: the first super-tile lands early
            # (matmuls can start), the rest come in three big transfers
            # (fewer descriptors and tile releases -> fewer semaphores).
            BATCH = [1, 5, 5, 5]
            k0 = 0
            fts = []          # (tile, batch_size)
            for g, bsz in enumerate(BATCH):
                ft = xpool.tile([128, bsz, 2, ROW], f8, name=f"ft{g}")
                nc.sync.dma_start(
                    ft[:, :, :, :],
                    x[k0 * 128:(k0 + bsz) * 128, :].rearrange(
                        "(b p) c -> p b c", b=bsz).rearrange(
                        "p b (i c) -> p b i c", i=2))
                fts.append((ft, bsz))
                k0 += bsz
            k = 0
            for ft, bsz in fts:
                for t in range(bsz):
                    for wi, (s, mc, nw) in enumerate(WINDOWS):
                        nc.tensor.matmul(
                            ps[wi][:, 0:nw],
                            ft[:, t, :, s * 128:(s + 1) * 128],
                            ft[:, t, :, mc:mc + nw],
                            start=(k == 0),
                            stop=(k == KT - 1),
                            perf_mode=DR,
                        )
                    k += 1

            ot = opool.tile([128, OUT_COLS], bf16)
            col = 0
            cuts = []
            for wi, (s, mc, nw) in enumerate(WINDOWS):
                if wi % 2 == 0:
                    nc.vector.tensor_copy(ot[:, col:col + nw], ps[wi][:, 0:nw])
                else:
                    nc.scalar.copy(ot[:, col:col + nw], ps[wi][:, 0:nw])
                col += nw
                if wi in (3, 7):
                    cuts.append(col)
            lo = 0
            for hi in cuts:
                nc.sync.dma_start(out[:, lo:hi], ot[:, lo:hi])
                lo = hi
    nc.compile()
    return nc


def _get_nc():
    if "nc" not in _CACHE:
        _CACHE["nc"] = _build()
    return _CACHE["nc"]


def _quantize(X, w):
    """Host prep: F = w^2 * X, scaled by a power of two into fp8e4 range."""
    F = (w.astype(np.float64) ** 2) * X.astype(np.float64)
    amax = float(np.abs(F).max())
    if amax == 0.0 or not np.isfinite(amax):
        scale = 1.0
    else:
        scale = 2.0 ** np.floor(np.log2(192.0 / amax))
    Fq = np.clip(F * scale, -240.0, 240.0).astype(ml_dtypes.float8_e4m3)
    return Fq, scale


def _in_maps(Fq):
    maps = []
    for units in ASSIGN:
        xc = np.concatenate([Fq[:, u * 128:(u + 1) * 128] for u in units],
                            axis=1)
        # interleave the two 128-row halves of each 256-row super-tile
        # into one partition line: [KT*128, 2*ROW]
        x2 = (xc.reshape(KT, 2, 128, ROW)
                .transpose(0, 2, 1, 3)
                .reshape(KT * 128, 2 * ROW))
        maps.append({"x": np.ascontiguousarray(x2)})
    return maps


def _assemble(outs, svec, scale):
    inv = 1.0 / (scale * scale)
    quad = {}
    for c, units in enumerate(ASSIGN):
        o = outs[c].astype(np.float64) * inv
        col = 0
        for (s, mc, nw) in WINDOWS:
            su = units[s]
            block = o[:, col:col + nw]
            col += nw
            m0 = mc // 128
            for t in range(nw // 128):
                quad[(su, units[m0 + t])] = block[:, t * 128:(t + 1) * 128]
    loss = 0.0
    for i in range(10):
        s_i = np.concatenate([svec[2 * i], svec[2 * i + 1]])
        for j in range(i + 1, 10):
            s_j = np.concatenate([svec[2 * j], svec[2 * j + 1]])
            A = np.empty((256, 256))
            for a in range(2):
                for b in range(2):
                    u, v = 2 * i + a, 2 * j + b
                    q = quad[(u, v)] if (u, v) in quad else quad[(v, u)].T
                    A[a * 128:(a + 1) * 128, b * 128:(b + 1) * 128] = q
            C = A - np.outer(s_i, s_j) / float(N)
            loss += float((C * C).sum())
    loss /= float((N - 1) * (N - 1))
    return np.asarray([loss], np.float32)


def kernel(final_readout, weight, _trace=False):
    X = np.ascontiguousarray(np.asarray(final_readout, np.float32))
    w = np.asarray(weight, np.float32)
    Fq, scale = _quantize(X, w)
    # column sums of the quantized data (exact, fp64) for the centering
    # correction; must match the data the device saw.
    scol = Fq.astype(np.float64).sum(axis=0) / scale
    svec = {u: scol[u * 128:(u + 1) * 128] for u in range(20)}
    nc = _get_nc()
    res = bass_utils.run_bass_kernel_spmd(
        nc, _in_maps(Fq), core_ids=list(range(8)), trace=_trace)
    _CACHE["last_results"] = res
    return _assemble([r["out"] for r in res.results], svec, scale)


# revision 17
# speedup vs baseline: 1.1342x; 1.1342x over previous
"""HSIC pairwise loss kernel for trn2 (8 NeuronCores), fp8 DoubleRow.

Math: reference builds K_c = (w^2 w^2T) * (E_c E_c^T), M_c = R K_c, and sums
tr(M_i M_j) over i<j. With F_c = w^2 * E_c (row scaling), R the centering
matrix (idempotent):
    tr(R K_i R K_j) = ||G_i^T G_j||_F^2,  G_c = F_c - colmean(F_c)
and with A_ij = F_i^T F_j, s_c = F_c^T 1:
    G_i^T G_j = A_ij - (1/n) s_i s_j^T
so loss = sum_{i<j} ||A_ij - s_i s_j^T / n||_F^2 / (n-1)^2.

Device work: the 45 A_ij blocks [256,256] (contraction over n=4096) at
half-chunk granularity (20 units of 128 cols). Each core loads 9 units
as fp8e4 (4.7 MB, host pre-scales w^2*X by a power-of-two and quantizes)
and runs 8 matmul windows per 256-row k-super-tile into 8 PSUM banks
using MatmulPerfMode.DoubleRow (2 fp8 k-rows per cycle). Column sums s
are taken on the host from the same quantized array, so the device does
matmuls only. Host assembles quadrants, applies the rank-1 centering
correction and the final scalar reduction in float64 (~3 MFLOP).
"""

import numpy as np
import ml_dtypes
from contextlib import ExitStack

import concourse.bass as bass
import concourse.tile as tile
from concourse import bacc, mybir
from concourse import bass_utils

N = 4096
KT = 16                      # k super-tiles of 256 rows (DoubleRow)
UNITS = 9                    # half-chunk units per core
ROW = UNITS * 128            # 1152 data cols
WARM_MMS = 24                # dummy matmuls to pre-warm the PE (HAM)

# 8 cores x 9 units (of 20 half-chunks); covers all 180 cross-parent
# half-pairs via the fixed window pattern below (found by search).
ASSIGN = [
    [0, 19, 9, 4, 3, 18, 13, 17, 1],
    [10, 5, 8, 18, 4, 12, 9, 16, 15],
    [5, 17, 12, 9, 18, 2, 7, 14, 11],
    [14, 7, 12, 11, 2, 13, 1, 15, 16],
    [0, 17, 3, 19, 4, 6, 12, 11, 15],
    [14, 2, 9, 16, 5, 3, 19, 0, 6],
    [17, 18, 13, 11, 1, 6, 5, 8, 10],
    [3, 2, 19, 0, 14, 4, 7, 10, 8],
]

# (stat_slot, moving_start_col, n_cols)
WINDOWS = [
    (0, 640, 512),
    (1, 640, 512),
    (2, 640, 512),
    (3, 640, 512),
    (4, 640, 512),
    (5, 768, 384),
    (6, 896, 256),
    (7, 1024, 128),
]
OUT_COLS = sum(w[2] for w in WINDOWS)   # 3328

_CACHE = {}


def _build():
    f32 = mybir.dt.float32
    f8 = mybir.dt.float8e4
    DR = mybir.MatmulPerfMode.DoubleRow
    nc = bacc.Bacc("TRN2", target_bir_lowering=False, debug=False,
                   num_devices=8)
    bf16 = mybir.dt.bfloat16
    # host pre-interleaves the two 128-row k-halves of each 256-row
    # super-tile into one 2304-byte partition line: x2[k*128+p, :] =
    # [F[k*256+p, :], F[k*256+128+p, :]] — DMA runs are contiguous.
    x = nc.dram_tensor("x", [KT * 128, 2 * ROW], f8,
                       kind="ExternalInput").ap()
    out = nc.dram_tensor("out", [128, OUT_COLS], bf16,
                         kind="ExternalOutput").ap()

    with tile.TileContext(nc) as tc:
        with ExitStack() as ctx:
            zpool = ctx.enter_context(tc.tile_pool(name="z", bufs=1))
            xpool = ctx.enter_context(tc.tile_pool(name="xs", bufs=1))
            psum = ctx.enter_context(tc.tile_pool(name="ps", bufs=1,
                                                  space="PSUM"))
            opool = ctx.enter_context(tc.tile_pool(name="o", bufs=1))

            ps = []
            for i, (_, _, nw) in enumerate(WINDOWS):
                pst = psum.tile([128, nw], f32, tag=f"ps{i}", name=f"ps{i}")
                ps.append(pst)

            # PE warm-up: dummy DoubleRow matmuls while the first input DMAs
            # are in flight (HAM needs ~3.4us of activity to unthrottle
            # 1.2 -> 2.4 GHz). Products land in ps[0] and are discarded
            # (the real start=True matmul resets it).
            zt = zpool.tile([128, 2, 128], f8, tag="zt")
            nc.vector.memset(zt[:], 0.0)
            for _ in range(WARM_MMS):
                nc.tensor.matmul(ps[0][:, 0:128], zt[:, :, :], zt[:, :, :],
                                 start=True, stop=True, perf_mode=DR)

            for k in range(KT):
                ft = xpool.tile([128, 2, ROW], f8, name=f"ft{k}")
                nc.sync.dma_start(
                    ft[:, :, :],
                    x[k * 128:(k + 1) * 128, :].rearrange(
                        "p (i c) -> p i c", i=2))
                for wi, (s, mc, nw) in enumerate(WINDOWS):
                    nc.tensor.matmul(
                        ps[wi][:, 0:nw],
                        ft[:, :, s * 128:(s + 1) * 128],
                        ft[:, :, mc:mc + nw],
                        start=(k == 0),
                        stop=(k == KT - 1),
                        perf_mode=DR,
                    )

            ot = opool.tile([128, OUT_COLS], bf16)
            col = 0
            cuts = []
            for wi, (s, mc, nw) in enumerate(WINDOWS):
                if wi % 2 == 0:
                    nc.vector.tensor_copy(ot[:, col:col + nw], ps[wi][:, 0:nw])
                else:
                    nc.scalar.copy(ot[:, col:col + nw], ps[wi][:, 0:nw])
                col += nw
                if wi in (3, 7):
                    cuts.append(col)
            lo = 0
            for hi in cuts:
                nc.sync.dma_start(out[:, lo:hi], ot[:, lo:hi])
                lo = hi
    nc.compile()
    return nc


def _get_nc():
    if "nc" not in _CACHE:
        _CACHE["nc"] = _build()
    return _CACHE["nc"]


def _quantize(X, w):
    """Host prep: F = w^2 * X, scaled by a power of two into fp8e4 range."""
    F = (w.astype(np.float64) ** 2) * X.astype(np.float64)
    amax = float(np.abs(F).max())
    if amax == 0.0 or not np.isfinite(amax):
        scale = 1.0
    else:
        scale = 2.0 ** np.floor(np.log2(192.0 / amax))
    Fq = np.clip(F * scale, -240.0, 240.0).astype(ml_dtypes.float8_e4m3)
    return Fq, scale


def _in_maps(Fq):
    maps = []
    for units in ASSIGN:
        xc = np.concatenate([Fq[:, u * 128:(u + 1) * 128] for u in units],
                            axis=1)
        # interleave the two 128-row halves of each 256-row super-tile
        # into one partition line: [KT*128, 2*ROW]
        x2 = (xc.reshape(KT, 2, 128, ROW)
                .transpose(0, 2, 1, 3)
                .reshape(KT * 128, 2 * ROW))
        maps.append({"x": np.ascontiguousarray(x2)})
    return maps


def _assemble(outs, svec, scale):
    inv = 1.0 / (scale * scale)
    quad = {}
    for c, units in enumerate(ASSIGN):
        o = outs[c].astype(np.float64) * inv
        col = 0
        for (s, mc, nw) in WINDOWS:
            su = units[s]
            block = o[:, col:col + nw]
            col += nw
            m0 = mc // 128
            for t in range(nw // 128):
                quad[(su, units[m0 + t])] = block[:, t * 128:(t + 1) * 128]
    loss = 0.0
    for i in range(10):
        s_i = np.concatenate([svec[2 * i], svec[2 * i + 1]])
        for j in range(i + 1, 10):
            s_j = np.concatenate([svec[2 * j], svec[2 * j + 1]])
            A = np.empty((256, 256))
            for a in range(2):
                for b in range(2):
                    u, v = 2 * i + a, 2 * j + b
                    q = quad[(u, v)] if (u, v) in quad else quad[(v, u)].T
                    A[a * 128:(a + 1) * 128, b * 128:(b + 1) * 128] = q
            C = A - np.outer(s_i, s_j) / float(N)
            loss += float((C * C).sum())
    loss /= float((N - 1) * (N - 1))
    return np.asarray([loss], np.float32)


def kernel(final_readout, weight, _trace=False):
    X = np.ascontiguousarray(np.asarray(final_readout, np.float32))
    w = np.asarray(weight, np.float32)
    Fq, scale = _quantize(X, w)
    # column sums of the quantized data (exact, fp64) for the centering
    # correction; must match the data the device saw.
    scol = Fq.astype(np.float64).sum(axis=0) / scale
    svec = {u: scol[u * 128:(u + 1) * 128] for u in range(20)}
    nc = _get_nc()
    res = bass_utils.run_bass_kernel_spmd(
        nc, _in_maps(Fq), core_ids=list(range(8)), trace=_trace)
    _CACHE["last_results"] = res
    return _assemble([r["out"] for r in res.results], svec, scale)
